# revision 2
# baseline (speedup 1.0000x reference)
"""ActiveRotatingFilter gather kernel for 8 Trainium2 NeuronCores.

Semantics (matching the reference):
    idx = indices.reshape(72, 8) - 1
    inv = argsort(idx, axis=0)   (stable)
    out[o, r, i, e] = input[o, i, inv[e, r]]      out: [O*R, I*nOri, kH, kW]

Strategy: shard O=512 across 8 cores (64 planes each). Per core the input
shard (4.7 MB) is loaded once into SBUF as [128 partitions = (o, i_hi),
9216 = (i_lo, e)]. For each of the 8 rotations the 72-entry permutation is
applied on-chip by VectorE copies (the ARF permutation factors into a
cyclic layer shift + a 9-element kernel permutation, giving <=18 strided
block copies per rotation; identity rotations skip the copy entirely),
then the permuted tile is written out with one fully-contiguous 4.7 MB DMA
per rotation. DMA-out (~105us/core) is the bottleneck; copies hide under it.
"""

import numpy as np
from contextlib import ExitStack

O, I, NORI, KH, KW = 512, 256, 8, 3, 3
R = 8
E = NORI * KH * KW          # 72
NCORES = 8
O_SH = O // NCORES          # 64 output planes per core
P = 128                     # SBUF partitions, p = o*2 + i_hi
IL = I // 2                 # 128 i_lo values per partition
FD = IL * E                 # 9216 f32 per partition
NB = 3                      # y-tile ring buffers

_cache = {}


def _plan_rotation(col):
    """Decompose one permutation column into block-copy ops.

    Returns a list of ops:
      ("lgroup", s, j, qj): for all l: dst (l, j) <- src ((l - s) % 8, qj)
      ("run", a, b, ln):    dst [a, a+ln) <- src [b, b+ln)
    """
    col = col.astype(int)
    layers = col.reshape(NORI, KH * KW) // (KH * KW)
    q = col.reshape(NORI, KH * KW) % (KH * KW)
    structured = all(np.all(layers[l] == layers[l][0]) for l in range(NORI))
    if structured:
        l0 = layers[:, 0]
        s = int((-l0[0]) % NORI)
        structured = np.array_equal(l0, (np.arange(NORI) - s) % NORI) and all(
            np.array_equal(q[l], q[0]) for l in range(NORI)
        )
    if structured:
        return [("lgroup", s, j, int(q[0][j])) for j in range(KH * KW)]
    ops = []
    e = 0
    while e < E:
        b = int(col[e])
        ln = 1
        while e + ln < E and col[e + ln] == b + ln:
            ln += 1
        ops.append(("run", e, b, ln))
        e += ln
    return ops


def _build(inv):
    import concourse.bass as bass
    import concourse.mybir as mybir

    f32 = mybir.dt.float32
    nc = bass.Bass("TRN2", target_bir_lowering=False, debug=False)
    x = nc.declare_dram_parameter("input", [P, FD], f32, isOutput=False)
    out = nc.declare_dram_parameter("out", [O_SH, R, 2, FD], f32, isOutput=True)

    ident = [r for r in range(R) if np.array_equal(inv[:, r], np.arange(E))]
    copies = [r for r in range(R) if r not in ident]
    out_order = ident + copies
    n_id = len(ident)
    rot_plans = {r: _plan_rotation(inv[:, r]) for r in copies}

    with ExitStack() as ctx:
        x_t = ctx.enter_context(nc.sbuf_tensor("x_t", [P, FD], f32))
        y_t = [
            ctx.enter_context(nc.sbuf_tensor(f"y_t{b}", [P, FD], f32))
            for b in range(NB)
        ]
        dma_sem = ctx.enter_context(nc.semaphore("dma_sem"))
        cp_sem = ctx.enter_context(nc.semaphore("cp_sem"))
        block = ctx.enter_context(nc.Block())

        @block.sync
        def _(sync):
            sync.dma_start(x_t[:], x[:]).then_inc(dma_sem, 16)
            sync.wait_ge(dma_sem, 16)
            for jo, r in enumerate(out_order):
                if jo < n_id:
                    sync.dma_start(out.ap()[:, r], x_t[:]).then_inc(dma_sem, 16)
                else:
                    k = jo - n_id
                    sync.wait_ge(cp_sem, k + 1)
                    sync.dma_start(out.ap()[:, r], y_t[k % NB][:]).then_inc(
                        dma_sem, 16
                    )
            sync.wait_ge(dma_sem, 16 * (1 + R))

        @block.vector
        def _(vector):
            vector.wait_ge(dma_sem, 16)
            x4 = x_t[:].rearrange("p (il l j) -> p il l j", il=IL, l=NORI)
            x3 = x_t[:].rearrange("p (il e) -> p il e", il=IL)
            for k, r in enumerate(copies):
                if k >= NB:
                    # wait for the out-DMA that last read this y buffer
                    vector.wait_ge(dma_sem, 16 * (n_id + (k - NB) + 2))
                yt = y_t[k % NB]
                y4 = yt[:].rearrange("p (il l j) -> p il l j", il=IL, l=NORI)
                y3 = yt[:].rearrange("p (il e) -> p il e", il=IL)
                pairs = []
                for op in rot_plans[r]:
                    if op[0] == "lgroup":
                        _, s, j, qj = op
                        if s == 0:
                            pairs.append((y4[:, :, :, j], x4[:, :, :, qj]))
                        else:
                            pairs.append(
                                (y4[:, :, s:NORI, j], x4[:, :, 0 : NORI - s, qj])
                            )
                            pairs.append(
                                (y4[:, :, 0:s, j], x4[:, :, NORI - s : NORI, qj])
                            )
                    else:
                        _, a, b, ln = op
                        pairs.append((y3[:, :, a : a + ln], x3[:, :, b : b + ln]))
                for i, (dst, src) in enumerate(pairs):
                    instr = vector.tensor_copy(dst, src)
                    if i == len(pairs) - 1:
                        instr.then_inc(cp_sem, 1)

    return nc


def kernel(input, indices):
    from concourse.bass_utils import run_bass_kernel_spmd

    input = np.ascontiguousarray(np.asarray(input), dtype=np.float32)
    indices = np.asarray(indices)
    assert input.shape == (O, I, NORI, KH, KW), input.shape
    idx = indices.reshape(E, R).astype(np.int64) - 1
    inv = np.argsort(idx, axis=0, kind="stable")

    key = inv.tobytes()
    if key not in _cache:
        _cache[key] = _build(inv)
    nc = _cache[key]

    xs = input.reshape(O, I * E)
    in_maps = [
        {"input": np.ascontiguousarray(xs[c * O_SH : (c + 1) * O_SH]).reshape(P, FD)}
        for c in range(NCORES)
    ]
    res = run_bass_kernel_spmd(nc, in_maps, core_ids=list(range(NCORES)))
    parts = [res.results[c]["out"].reshape(O_SH, R, I, E) for c in range(NCORES)]
    full = np.concatenate(parts, axis=0)           # [O, R, I, E]
    return full.reshape(O * R, I * NORI, KH, KW)
